# revision 17
# baseline (speedup 1.0000x reference)
"""Trainium2 Bass kernel for nn_AttDual: dual-attention MIL pooling head.

Computation (see reference):
  K = chain(features, key_*)    ; chain = Linear -> LayerNorm -> GELU(erf)
  V = chain(K, value_*)
  Q = chain(K, query_*)
  top_idx = argmax(c, axis=0)   ; q_max = chain(K[top_idx], query_*)  (== Q[top_idx])
  A = softmax(Q @ q_max.T / 32, axis=0)   (column softmax over all N)
  B = A.T @ V ;  C = einsum('ik,oik->o', B, head_w) + head_b
Returns (C [1,7], A [N,7], B [1,7,1024]).

Distribution: data-parallel over N across 8 NeuronCores. Three tiny
AllReduces: column max of c (argmax selection), selected K rows, and
[colsum | B] for the softmax normalizer + B.

Matmuls in bf16 with f32 PSUM accumulation. LayerNorm stats via bn_stats;
rsqrt via DVE bit-trick + Newton (keeps ScalarE on the Gelu table the whole
main pass; softmax exp deferred to the tail = 1 table switch total).
"""
import numpy as np

import concourse.bacc as bacc
import concourse.tile as tile
import concourse.mybir as mybir
from concourse.bass_utils import run_bass_kernel_spmd

F32 = mybir.dt.float32
I32 = mybir.dt.int32
BF16 = mybir.dt.bfloat16
AF = mybir.ActivationFunctionType
ALU = mybir.AluOpType

N_CORES = 8
N_TOT = 50000
D = 1024
DK = 8          # d / 128
C7 = 7
EPS = 1e-5
NEG_BIG = -1.0e30

_BUILD_CACHE: dict = {}


# --------------------------------------------------------------------------
# graph builder
# --------------------------------------------------------------------------

def build_graph(rs: int, n_cores: int = N_CORES):
    """rs = rows per core (real). Tiles of 128 rows; last tile partial."""
    rt = (rs + 127) // 128
    vr_last = rs - (rt - 1) * 128

    nc = bacc.Bacc("TRN2", target_bir_lowering=False, debug=False,
                   num_devices=n_cores)

    xt_ext = nc.declare_dram_parameter("xt", [rt, 128, DK, 128], BF16, isOutput=False)
    c_ext = nc.declare_dram_parameter("call", [128, rt, C7], F32, isOutput=False)
    wk_ext = nc.declare_dram_parameter("wk", [DK, 128, D], BF16, isOutput=False)
    wv_ext = nc.declare_dram_parameter("wv", [DK, 128, D], BF16, isOutput=False)
    wq_ext = nc.declare_dram_parameter("wq", [DK, 128, D], BF16, isOutput=False)
    kb_ext = nc.declare_dram_parameter("kb", [1, D], BF16, isOutput=False)
    vb_ext = nc.declare_dram_parameter("vb", [1, D], BF16, isOutput=False)
    qb_ext = nc.declare_dram_parameter("qb", [1, D], BF16, isOutput=False)
    hwt_ext = nc.declare_dram_parameter("hwt", [128, DK, C7, C7], F32, isOutput=False)
    hb_ext = nc.declare_dram_parameter("hb", [1, C7], F32, isOutput=False)

    a_ext = nc.declare_dram_parameter("A_out", [128, rt, C7], F32, isOutput=True)
    b_ext = nc.declare_dram_parameter("B_out", [C7, D], F32, isOutput=True)
    co_ext = nc.declare_dram_parameter("C_out", [1, C7], F32, isOutput=True)

    rg = [list(range(n_cores))]

    with tile.TileContext(nc) as tc:
        with (
            tc.tile_pool(name="const", bufs=1) as cp,
            tc.tile_pool(name="io", bufs=2) as iop,
            tc.tile_pool(name="act", bufs=2) as ap_,
            tc.tile_pool(name="k1tp", bufs=4) as k1tp,
            tc.tile_pool(name="sm", bufs=2) as smp,
            tc.tile_pool(name="ps_h", bufs=2, space="PSUM") as ps_h,
            tc.tile_pool(name="ps_acc", bufs=1, space="PSUM") as ps_acc,
            tc.tile_pool(name="ps_sm", bufs=2, space="PSUM") as ps_sm,
            tc.tile_pool(name="dram", bufs=1, space="DRAM") as dram,
        ):
            # ---------------- constants / pinned ----------------
            c_all = cp.tile([128, rt, C7], F32)
            nc.sync.dma_start(c_all[:], c_ext[:, :, :])
            wk_sb = [cp.tile([128, D], BF16, tag=f"wk{k}", name=f"wk_sb{k}") for k in range(DK)]
            wv_sb = [cp.tile([128, D], BF16, tag=f"wv{k}", name=f"wv_sb{k}") for k in range(DK)]
            wq_sb = [cp.tile([128, D], BF16, tag=f"wq{k}", name=f"wq_sb{k}") for k in range(DK)]
            for k in range(DK):
                nc.sync.dma_start(wk_sb[k][:], wk_ext.ap()[k])
            for k in range(DK):
                nc.sync.dma_start(wv_sb[k][:], wv_ext.ap()[k])
            for k in range(DK):
                nc.sync.dma_start(wq_sb[k][:], wq_ext.ap()[k])
            bias_sb = cp.tile([65, D], BF16)
            nc.sync.dma_start(bias_sb[0:1, :], kb_ext[:, :])
            nc.sync.dma_start(bias_sb[32:33, :], vb_ext[:, :])
            nc.sync.dma_start(bias_sb[64:65, :], qb_ext[:, :])
            kb_sb, vb_sb, qb_sb = bias_sb[0:1, :], bias_sb[32:33, :], bias_sb[64:65, :]
            hwt_sb = cp.tile([128, DK, C7, C7], F32)
            nc.sync.dma_start(hwt_sb[:], hwt_ext[:, :, :, :])
            hb_sb = cp.tile([1, C7], F32)
            nc.sync.dma_start(hb_sb[:], hb_ext[:, :])

            v_full = cp.tile([128, rt, D], BF16)
            p_all = cp.tile([128, rt, C7], BF16)
            a_full = cp.tile([128, rt, C7], F32)

            ones_bf = cp.tile([65, 128], BF16)
            nc.gpsimd.memset(ones_bf[:], 1.0)
            ones_col = cp.tile([128, 1], BF16)
            nc.gpsimd.memset(ones_col[:], 1.0)
            one_f = cp.tile([1, 1], F32)
            nc.gpsimd.memset(one_f[:], 1.0)

            # identity matrices (via iota + is_equal)
            iota_p = cp.tile([128, 1], I32)
            nc.gpsimd.iota(iota_p[:], [[0, 1]], channel_multiplier=1)
            iota_pf = cp.tile([128, 1], F32)
            nc.vector.tensor_copy(iota_pf[:], iota_p[:])
            iota_f = cp.tile([128, 128], I32)
            nc.gpsimd.iota(iota_f[:], [[1, 128]], channel_multiplier=0)
            iota_ff = cp.tile([128, 128], F32)
            nc.vector.tensor_copy(iota_ff[:], iota_f[:])
            ident_f = cp.tile([128, 128], F32)
            nc.vector.tensor_scalar(out=ident_f[:], in0=iota_ff[:], scalar1=iota_pf[:],
                                    scalar2=None, op0=ALU.is_equal)
            ident_bf = cp.tile([128, 128], BF16)
            nc.vector.tensor_copy(ident_bf[:], ident_f[:])

            # collective bounce buffers
            mx_in = dram.tile([1, C7], F32)
            mx_out = dram.tile([1, C7], F32, addr_space="Shared")
            ms_in = dram.tile([C7, D], F32)
            ms_out = dram.tile([C7, D], F32, addr_space="Shared")
            red_in = dram.tile([C7, 1 + D], F32)
            red_out = dram.tile([C7, 1 + D], F32, addr_space="Shared")
            qt_dram = dram.tile([rt, 128, DK, 128], BF16)

            # ---------------- pass 0: column max of c ----------------
            cmax = cp.tile([128, C7], F32)
            nc.vector.tensor_reduce(out=cmax[:], in_=c_all[:].rearrange("p t j -> p j t"),
                                    axis=mybir.AxisListType.X, op=ALU.max)
            ctp = ps_sm.tile([C7, 128], F32, tag="sm")
            nc.tensor.transpose(ctp[:], cmax[:], ident_f[:])
            mloc = cp.tile([C7, 1], F32)
            nc.vector.tensor_reduce(out=mloc[:], in_=ctp[:], axis=mybir.AxisListType.X,
                                    op=ALU.max)
            nc.sync.dma_start(mx_in[:], mloc[:])
            nc.gpsimd.collective_compute(
                "AllReduce", ALU.max, replica_groups=rg,
                ins=[mx_in[:].opt()], outs=[mx_out[:].opt()])
            mg1 = cp.tile([1, C7], F32)
            nc.sync.dma_start(mg1[:], mx_out[:])
            mg128 = cp.tile([128, C7], F32)
            nc.gpsimd.partition_broadcast(mg128[:], mg1[:])

            # ---------------- helpers ----------------
            def rsqrt_cols(vv, y, t1, t2, p, w):
                """y[:p, :w] = rsqrt(vv[:p, :w]) via bit trick + 2 Newton."""
                nc.vector.tensor_scalar(out=y[:p, :w].bitcast(I32),
                                        in0=vv[:p, :w].bitcast(I32),
                                        scalar1=1, scalar2=-1,
                                        op0=ALU.logical_shift_right,
                                        op1=ALU.bitwise_xor)
                nc.vector.tensor_scalar(out=y[:p, :w].bitcast(I32),
                                        in0=y[:p, :w].bitcast(I32),
                                        scalar1=0x5F3759E0, scalar2=None, op0=ALU.add)
                for _ in range(2):
                    nc.vector.tensor_tensor(out=t1[:p, :w], in0=y[:p, :w],
                                            in1=y[:p, :w], op=ALU.mult)
                    nc.vector.tensor_tensor(out=t2[:p, :w], in0=t1[:p, :w],
                                            in1=vv[:p, :w], op=ALU.mult)
                    nc.vector.tensor_scalar(out=t2[:p, :w], in0=t2[:p, :w], scalar1=-0.5,
                                            scalar2=1.5, op0=ALU.mult, op1=ALU.add)
                    nc.vector.tensor_tensor(out=y[:p, :w], in0=y[:p, :w],
                                            in1=t2[:p, :w], op=ALU.mult)

            def ln_stats(h_ps, mv, p, tagp):
                """mv[:p, 0:2] = (mean, var) of h_ps[:p, :D]."""
                bns = smp.tile([128, 12], F32, tag=f"bns{tagp}")
                nc.vector.bn_stats(bns[:p, 0:6], h_ps[:p, 0:512])
                nc.vector.bn_stats(bns[:p, 6:12], h_ps[:p, 512:1024])
                nc.vector.bn_aggr(mv[:p, :], bns[:p, :])

            def ln_gelu(h_ps, out_ap, p, tagp):
                """out = gelu((h - mean)/sqrt(var+eps)); h_ps [p, D] psum f32."""
                mv = smp.tile([128, 2], F32, tag=f"mv{tagp}")
                vv = smp.tile([128, 1], F32, tag=f"vv{tagp}")
                y = smp.tile([128, 1], F32, tag=f"y{tagp}")
                t1 = smp.tile([128, 1], F32, tag=f"t1{tagp}")
                t2 = smp.tile([128, 1], F32, tag=f"t2{tagp}")
                nb = smp.tile([128, 1], F32, tag=f"nb{tagp}")
                ln_stats(h_ps, mv, p, tagp)
                nc.vector.tensor_scalar(out=vv[:p, :], in0=mv[:p, 1:2], scalar1=EPS,
                                        scalar2=None, op0=ALU.add)
                rsqrt_cols(vv, y, t1, t2, p, 1)
                nc.vector.scalar_tensor_tensor(out=nb[:p, :], in0=mv[:p, 0:1],
                                               scalar=-1.0, in1=y[:p, :],
                                               op0=ALU.mult, op1=ALU.mult)
                nc.scalar.activation(out_ap, h_ps[:p, :], AF.Gelu,
                                     bias=nb[:p, :], scale=y[:p, :])

            def chain_mms(h_ps, lhs_fn, w_sb, b_sb, p):
                """h_ps[:p, :] = lhsT.T @ W + bias (accumulated over DK k-tiles)."""
                for oc in range(2):
                    sl = slice(oc * 512, (oc + 1) * 512)
                    for k in range(DK):
                        nc.tensor.matmul(h_ps[:p, sl], lhs_fn(k), w_sb[k][:, sl],
                                         start=(k == 0), stop=False)
                    nc.tensor.matmul(h_ps[:p, sl],
                                     ones_bf[b_sb.base_partition():b_sb.base_partition() + 1, :p],
                                     b_sb[:1, sl],
                                     start=False, stop=True)

            # shared sequential psum accumulator slot (msel -> hq -> bp)
            msel_ps = ps_acc.tile([C7, D], F32, tag="acc")

            # ---------------- main pass (software-pipelined by 1 tile) ------
            k1_tiles = {}
            k1t_tiles = {}

            def emit_k_chain(t):
                vr = 128  # chains run on full tiles; garbage rows are masked later
                xt_t = iop.tile([128, DK, 128], BF16, tag="xt")
                nc.gpsimd.dma_start(xt_t[:], xt_ext.ap()[t])
                h_ps = ps_h.tile([128, D], F32, tag="h")
                chain_mms(h_ps, lambda k: xt_t[:, k, :], wk_sb, kb_sb, vr)
                k1 = ap_.tile([128, D], BF16, tag="k1")
                ln_gelu(h_ps, k1[:], vr, "m")
                # m_sel accumulation (valid rows only)
                pvr = vr_last if t == rt - 1 else 128
                oh = smp.tile([128, C7], BF16, tag="oh")
                nc.vector.tensor_tensor(out=oh[:], in0=c_all[:, t, :], in1=mg128[:],
                                        op=ALU.is_equal)
                for oc in range(2):
                    sl = slice(oc * 512, (oc + 1) * 512)
                    nc.tensor.matmul(msel_ps[:, sl], oh[:pvr, :], k1[:pvr, sl],
                                     start=(t == 0), stop=(t == rt - 1))
                # K1^T via one batched DMA transpose: k1t[p,k,r] = k1[r, k*128+p]
                k1t = k1tp.tile([128, DK, 128], BF16, tag="k1t")
                nc.sync.dma_start(k1t[:], k1[:], transpose=True)
                k1_tiles[t] = k1
                k1t_tiles[t] = k1t

            def emit_vq_chains(t):
                k1t = k1t_tiles.pop(t)
                k1_tiles.pop(t)
                # V and Q chains with shared stats/rsqrt work
                h_v = ps_h.tile([128, D], F32, tag="h")
                chain_mms(h_v, lambda k: k1t[:, k, :], wv_sb, vb_sb, 128)
                h_q = ps_h.tile([128, D], F32, tag="h")
                chain_mms(h_q, lambda k: k1t[:, k, :], wq_sb, qb_sb, 128)
                mv_v = smp.tile([128, 2], F32, tag="mvv")
                mv_q = smp.tile([128, 2], F32, tag="mvq")
                ln_stats(h_v, mv_v, 128, "v")
                ln_stats(h_q, mv_q, 128, "q2")
                vv = smp.tile([128, 2], F32, tag="vv2")
                y = smp.tile([128, 2], F32, tag="y2")
                t1 = smp.tile([128, 2], F32, tag="t12")
                t2 = smp.tile([128, 2], F32, tag="t22")
                nb = smp.tile([128, 2], F32, tag="nb2")
                nc.vector.tensor_scalar(out=vv[:, 0:1], in0=mv_v[:, 1:2], scalar1=EPS,
                                        scalar2=None, op0=ALU.add)
                nc.vector.tensor_scalar(out=vv[:, 1:2], in0=mv_q[:, 1:2], scalar1=EPS,
                                        scalar2=None, op0=ALU.add)
                rsqrt_cols(vv, y, t1, t2, 128, 2)
                nc.vector.scalar_tensor_tensor(out=nb[:, 0:1], in0=mv_v[:, 0:1],
                                               scalar=-1.0, in1=y[:, 0:1],
                                               op0=ALU.mult, op1=ALU.mult)
                nc.vector.scalar_tensor_tensor(out=nb[:, 1:2], in0=mv_q[:, 0:1],
                                               scalar=-1.0, in1=y[:, 1:2],
                                               op0=ALU.mult, op1=ALU.mult)
                nc.scalar.activation(v_full[:, t, :], h_v[:, :], AF.Gelu,
                                     bias=nb[:, 0:1], scale=y[:, 0:1])
                q1 = ap_.tile([128, D], BF16, tag="q1")
                nc.scalar.activation(q1[:], h_q[:, :], AF.Gelu,
                                     bias=nb[:, 1:2], scale=y[:, 1:2])
                qt = ap_.tile([128, DK, 128], BF16, tag="qt")
                nc.sync.dma_start(qt[:], q1[:], transpose=True)
                nc.gpsimd.dma_start(qt_dram[t], qt[:])

            LAG = 1
            for t in range(rt):
                emit_k_chain(t)
                if t >= LAG:
                    emit_vq_chains(t - LAG)
            for t in range(max(0, rt - LAG), rt):
                emit_vq_chains(t)

            # ---------------- m_sel reduce + q_max chain ----------------
            msel_sb = cp.tile([C7, D], F32)
            nc.vector.tensor_copy(msel_sb[:], msel_ps[:])
            nc.sync.dma_start(ms_in[:], msel_sb[:])
            nc.gpsimd.collective_compute(
                "AllReduce", ALU.add, replica_groups=rg,
                ins=[ms_in[:].opt()], outs=[ms_out[:].opt()])
            msel_red = msel_sb
            nc.sync.dma_start(msel_red[:], ms_out[:])

            mselT = cp.tile([128, DK, C7], BF16)
            for k in range(DK):
                tp = ps_sm.tile([128, 128], F32, tag="sm")
                nc.tensor.transpose(tp[:, 0:C7], msel_red[:, k * 128:(k + 1) * 128],
                                    ident_f[0:C7, 0:C7])
                nc.vector.tensor_copy(mselT[:, k, :], tp[:, 0:C7])

            hq_ps = ps_acc.tile([C7, D], F32, tag="acc")
            chain_mms(hq_ps, lambda k: mselT[:, k, :], wq_sb, qb_sb, C7)
            qmax = cp.tile([C7, D], BF16)
            ln_gelu(hq_ps, qmax[:], C7, "q")
            qmaxT = cp.tile([128, DK, C7], BF16)
            for k in range(DK):
                tpb = ps_sm.tile([128, 128], BF16, tag="sm")
                nc.tensor.transpose(tpb[:, 0:C7], qmax[:, k * 128:(k + 1) * 128],
                                    ident_bf[0:C7, 0:C7])
                nc.vector.tensor_copy(qmaxT[:, k, :], tpb[:, 0:C7])

            # ---------------- tail: logits -> exp -> colsum/Bp ----------------
            bp_ps = ps_acc.tile([C7, D], F32, tag="acc")
            cs_acc = cp.tile([C7, 2], F32)
            nc.gpsimd.memset(cs_acc[:], 0.0)
            for t in range(rt):
                pvr = vr_last if t == rt - 1 else 128
                qt_t = iop.tile([128, DK, 128], BF16, tag="qtl")
                nc.gpsimd.dma_start(qt_t[:], qt_dram[t])
                lg = ps_sm.tile([C7, 128], F32, tag="sm")
                for k in range(DK):
                    nc.tensor.matmul(lg[:], qmaxT[:, k, :], qt_t[:, k, :],
                                     start=(k == 0), stop=(k == DK - 1))
                pt_t = smp.tile([C7, 128], BF16, tag="ptt")
                nc.scalar.activation(pt_t[:], lg[:], AF.Exp,
                                     scale=float(1.0 / np.sqrt(D)))
                nc.vector.tensor_reduce(out=cs_acc[:, 1:2], in_=pt_t[:, 0:pvr],
                                        axis=mybir.AxisListType.X, op=ALU.add)
                nc.vector.tensor_tensor(out=cs_acc[:, 0:1], in0=cs_acc[:, 0:1],
                                        in1=cs_acc[:, 1:2], op=ALU.add)
                ptp = ps_sm.tile([128, 128], BF16, tag="sm")
                nc.tensor.transpose(ptp[:, 0:C7], pt_t[:],
                                    ident_bf[0:C7, 0:C7])
                nc.vector.tensor_copy(p_all[:, t, :], ptp[:, 0:C7])
                for oc in range(2):
                    sl = slice(oc * 512, (oc + 1) * 512)
                    nc.tensor.matmul(bp_ps[:, sl], p_all[:pvr, t, :],
                                     v_full[:pvr, t, sl],
                                     start=(t == 0), stop=(t == rt - 1))

            # ---------------- final reduce: [colsum | Bp] ----------------
            red_sb = cp.tile([C7, 1 + D], F32)
            nc.vector.tensor_copy(red_sb[:, 0:1], cs_acc[:, 0:1])
            nc.vector.tensor_copy(red_sb[:, 1:1 + D], bp_ps[:])
            nc.sync.dma_start(red_in[:], red_sb[:])
            nc.gpsimd.collective_compute(
                "AllReduce", ALU.add, replica_groups=rg,
                ins=[red_in[:].opt()], outs=[red_out[:].opt()])
            red2 = red_sb
            nc.sync.dma_start(red2[:], red_out[:])

            # B = Bp / denom (per class)
            rden = cp.tile([C7, 1], F32)
            nc.vector.reciprocal(rden[:], red2[:, 0:1])
            b_sb = cp.tile([C7, D], F32)
            nc.vector.tensor_scalar(out=b_sb[:], in0=red2[:, 1:1 + D], scalar1=rden[:],
                                    scalar2=None, op0=ALU.mult)
            nc.sync.dma_start(b_ext[:, :], b_sb[:])

            # A = P / denom  (broadcast recip over rows)
            den1 = cp.tile([1, C7], F32)
            nc.sync.dma_start(den1[:], red_out[:].rearrange("j o -> o j")[0:1, :])
            rden1 = cp.tile([1, C7], F32)
            nc.vector.reciprocal(rden1[:], den1[:])
            rb128 = cp.tile([128, C7], F32)
            nc.gpsimd.partition_broadcast(rb128[:], rden1[:])
            for t in range(rt):
                pf = smp.tile([128, C7], F32, tag="pf")
                nc.vector.tensor_copy(pf[:], p_all[:, t, :])
                nc.vector.tensor_tensor(out=a_full[:, t, :], in0=pf[:], in1=rb128[:],
                                        op=ALU.mult)
            nc.sync.dma_start(a_ext[:, :, :], a_full[:])

            # C = einsum(B, head_w) + head_b
            btf = cp.tile([128, DK, C7], F32)
            for k in range(DK):
                tp = ps_sm.tile([128, 128], F32, tag="sm")
                nc.tensor.transpose(tp[:, 0:C7], b_sb[:, k * 128:(k + 1) * 128],
                                    ident_f[0:C7, 0:C7])
                nc.vector.tensor_copy(btf[:, k, :], tp[:, 0:C7])
            c_ps = ps_sm.tile([1, C7], F32, tag="sm")
            first = True
            for kb in range(DK):
                for i in range(C7):
                    nc.tensor.matmul(c_ps[:], btf[:, kb, i:i + 1], hwt_sb[:, kb, i, :],
                                     start=first, stop=False)
                    first = False
            nc.tensor.matmul(c_ps[:], one_f[:], hb_sb[:], start=False, stop=True)
            c_sb = cp.tile([1, C7], F32)
            nc.vector.tensor_copy(c_sb[:], c_ps[:])
            nc.sync.dma_start(co_ext[:, :], c_sb[:])

    nc.compile()
    return nc


# --------------------------------------------------------------------------
# host-side prep / unshard
# --------------------------------------------------------------------------

def _prep_core_inputs(shard_x, shard_c, rt):
    """shard_x [rs, 1024] f32, shard_c [rs, 7] f32 -> device layouts."""
    import ml_dtypes
    rs = shard_x.shape[0]
    rpad = rt * 128
    xp = np.zeros((rpad, D), dtype=np.float32)
    xp[:rs] = shard_x
    # [t, p(i_in_tile), k(i_tile), r] ; value = x[t*128+r, k*128+p]
    xt = np.ascontiguousarray(
        xp.reshape(rt, 128, DK, 128).transpose(0, 3, 2, 1)
    ).astype(ml_dtypes.bfloat16)
    cp_ = np.full((rpad, C7), NEG_BIG, dtype=np.float32)
    cp_[:rs] = shard_c
    call = np.ascontiguousarray(cp_.reshape(rt, 128, C7).transpose(1, 0, 2))
    return xt, call


def _prep_shared_inputs(inp):
    import ml_dtypes
    bf = ml_dtypes.bfloat16
    out = {}
    for nm, key in (("wk", "key_w"), ("wv", "value_w"), ("wq", "query_w")):
        w = np.asarray(inp[key], dtype=np.float32)
        out[nm] = np.ascontiguousarray(w.T.reshape(DK, 128, D)).astype(bf)
    for nm, key in (("kb", "key_b"), ("vb", "value_b"), ("qb", "query_b")):
        out[nm] = np.asarray(inp[key], dtype=np.float32).reshape(1, D).astype(bf)
    hw = np.asarray(inp["head_w"], dtype=np.float32)  # [7, 7, 1024]
    out["hwt"] = np.ascontiguousarray(
        hw.transpose(2, 1, 0).reshape(DK, 128, C7, C7).transpose(1, 0, 2, 3)
    )
    out["hb"] = np.asarray(inp["head_b"], dtype=np.float32).reshape(1, C7)
    return out


def kernel(**inputs) -> tuple:
    feats = np.asarray(inputs["features"], dtype=np.float32)
    c = np.asarray(inputs["c"], dtype=np.float32)
    n = feats.shape[0]
    assert n % N_CORES == 0
    rs = n // N_CORES
    rt = (rs + 127) // 128

    key = (rs, N_CORES)
    if key not in _BUILD_CACHE:
        _BUILD_CACHE[key] = build_graph(rs, N_CORES)
    nc = _BUILD_CACHE[key]

    shared = _prep_shared_inputs(inputs)
    in_maps = []
    for s in range(N_CORES):
        xt, call = _prep_core_inputs(feats[s * rs:(s + 1) * rs],
                                     c[s * rs:(s + 1) * rs], rt)
        m = {"xt": xt, "call": call}
        m.update(shared)
        in_maps.append(m)

    res = run_bass_kernel_spmd(nc, in_maps, core_ids=list(range(N_CORES)))

    a_parts = []
    for s in range(N_CORES):
        a_po = res.results[s]["A_out"]                      # [128, rt, 7]
        a_parts.append(a_po.transpose(1, 0, 2).reshape(rt * 128, C7)[:rs])
    A = np.concatenate(a_parts, axis=0).astype(np.float32)
    B = res.results[0]["B_out"].reshape(1, C7, D).astype(np.float32)
    Cc = res.results[0]["C_out"].reshape(1, C7).astype(np.float32)
    return (Cc, A, B)


# revision 18
# speedup vs baseline: 1.3294x; 1.3294x over previous
"""Trainium2 Bass kernel for nn_AttDual: dual-attention MIL pooling head.

Computation (see reference):
  K = chain(features, key_*)    ; chain = Linear -> LayerNorm -> GELU(erf)
  V = chain(K, value_*)
  Q = chain(K, query_*)
  top_idx = argmax(c, axis=0)   ; q_max = chain(K[top_idx], query_*)  (== Q[top_idx])
  A = softmax(Q @ q_max.T / 32, axis=0)   (column softmax over all N)
  B = A.T @ V ;  C = einsum('ik,oik->o', B, head_w) + head_b
Returns (C [1,7], A [N,7], B [1,7,1024]).

Distribution: data-parallel over N across 8 NeuronCores. Three tiny
AllReduces: column max of c (argmax selection), selected K rows, and
[colsum | B] for the softmax normalizer + B.

Matmuls in bf16 with f32 PSUM accumulation. LayerNorm stats via bn_stats;
rsqrt via DVE bit-trick + Newton (keeps ScalarE on the Gelu table the whole
main pass; softmax exp deferred to the tail = 1 table switch total).
"""
import numpy as np

import concourse.bacc as bacc
import concourse.tile as tile
import concourse.mybir as mybir
from concourse.bass_utils import run_bass_kernel_spmd

F32 = mybir.dt.float32
I32 = mybir.dt.int32
BF16 = mybir.dt.bfloat16
AF = mybir.ActivationFunctionType
ALU = mybir.AluOpType

N_CORES = 8
N_TOT = 50000
D = 1024
DK = 8          # d / 128
C7 = 7
EPS = 1e-5
NEG_BIG = -1.0e30

_BUILD_CACHE: dict = {}


# --------------------------------------------------------------------------
# graph builder
# --------------------------------------------------------------------------

def build_graph(rs: int, n_cores: int = N_CORES):
    """rs = rows per core (real). Tiles of 128 rows; last tile partial."""
    rt = (rs + 127) // 128
    vr_last = rs - (rt - 1) * 128

    nc = bacc.Bacc("TRN2", target_bir_lowering=False, debug=False,
                   num_devices=n_cores)

    xt_ext = nc.declare_dram_parameter("xt", [rt, 128, DK, 128], BF16, isOutput=False)
    c_ext = nc.declare_dram_parameter("call", [128, rt, C7], F32, isOutput=False)
    wk_ext = nc.declare_dram_parameter("wk", [DK, 128, D], BF16, isOutput=False)
    wv_ext = nc.declare_dram_parameter("wv", [DK, 128, D], BF16, isOutput=False)
    wq_ext = nc.declare_dram_parameter("wq", [DK, 128, D], BF16, isOutput=False)
    kb_ext = nc.declare_dram_parameter("kb", [1, D], BF16, isOutput=False)
    vb_ext = nc.declare_dram_parameter("vb", [1, D], BF16, isOutput=False)
    qb_ext = nc.declare_dram_parameter("qb", [1, D], BF16, isOutput=False)
    hwt_ext = nc.declare_dram_parameter("hwt", [128, DK, C7, C7], F32, isOutput=False)
    hb_ext = nc.declare_dram_parameter("hb", [1, C7], F32, isOutput=False)

    a_ext = nc.declare_dram_parameter("A_out", [128, rt, C7], F32, isOutput=True)
    b_ext = nc.declare_dram_parameter("B_out", [C7, D], F32, isOutput=True)
    co_ext = nc.declare_dram_parameter("C_out", [1, C7], F32, isOutput=True)

    rg = [list(range(n_cores))]

    with tile.TileContext(nc) as tc:
        with (
            tc.tile_pool(name="const", bufs=1) as cp,
            tc.tile_pool(name="io", bufs=2) as iop,
            tc.tile_pool(name="act", bufs=2) as ap_,
            tc.tile_pool(name="k1tp", bufs=4) as k1tp,
            tc.tile_pool(name="sm", bufs=2) as smp,
            tc.tile_pool(name="ps_h", bufs=2, space="PSUM") as ps_h,
            tc.tile_pool(name="ps_acc", bufs=1, space="PSUM") as ps_acc,
            tc.tile_pool(name="ps_sm", bufs=2, space="PSUM") as ps_sm,
            tc.tile_pool(name="dram", bufs=1, space="DRAM") as dram,
        ):
            # ---------------- constants / pinned ----------------
            c_all = cp.tile([128, rt, C7], F32)
            nc.sync.dma_start(c_all[:], c_ext[:, :, :])
            wk_sb = [cp.tile([128, D], BF16, tag=f"wk{k}", name=f"wk_sb{k}") for k in range(DK)]
            wv_sb = [cp.tile([128, D], BF16, tag=f"wv{k}", name=f"wv_sb{k}") for k in range(DK)]
            wq_sb = [cp.tile([128, D], BF16, tag=f"wq{k}", name=f"wq_sb{k}") for k in range(DK)]
            for k in range(DK):
                nc.sync.dma_start(wk_sb[k][:], wk_ext.ap()[k])
            for k in range(DK):
                nc.sync.dma_start(wv_sb[k][:], wv_ext.ap()[k])
            for k in range(DK):
                nc.sync.dma_start(wq_sb[k][:], wq_ext.ap()[k])
            bias_sb = cp.tile([65, D], BF16)
            nc.sync.dma_start(bias_sb[0:1, :], kb_ext[:, :])
            nc.sync.dma_start(bias_sb[32:33, :], vb_ext[:, :])
            nc.sync.dma_start(bias_sb[64:65, :], qb_ext[:, :])
            kb_sb, vb_sb, qb_sb = bias_sb[0:1, :], bias_sb[32:33, :], bias_sb[64:65, :]
            hwt_sb = cp.tile([128, DK, C7, C7], F32)
            nc.sync.dma_start(hwt_sb[:], hwt_ext[:, :, :, :])
            hb_sb = cp.tile([1, C7], F32)
            nc.sync.dma_start(hb_sb[:], hb_ext[:, :])

            v_full = cp.tile([128, rt, D], BF16)
            p_all = cp.tile([128, rt, C7], BF16)
            a_full = cp.tile([128, rt, C7], F32)

            ones_bf = cp.tile([65, 128], BF16)
            nc.gpsimd.memset(ones_bf[:], 1.0)
            ones_col = cp.tile([128, 1], BF16)
            nc.gpsimd.memset(ones_col[:], 1.0)
            one_f = cp.tile([1, 1], F32)
            nc.gpsimd.memset(one_f[:], 1.0)

            # identity matrices (via iota + is_equal)
            iota_p = cp.tile([128, 1], I32)
            nc.gpsimd.iota(iota_p[:], [[0, 1]], channel_multiplier=1)
            iota_pf = cp.tile([128, 1], F32)
            nc.vector.tensor_copy(iota_pf[:], iota_p[:])
            iota_f = cp.tile([128, 128], I32)
            nc.gpsimd.iota(iota_f[:], [[1, 128]], channel_multiplier=0)
            iota_ff = cp.tile([128, 128], F32)
            nc.vector.tensor_copy(iota_ff[:], iota_f[:])
            ident_f = cp.tile([128, 128], F32)
            nc.vector.tensor_scalar(out=ident_f[:], in0=iota_ff[:], scalar1=iota_pf[:],
                                    scalar2=None, op0=ALU.is_equal)
            ident_bf = cp.tile([128, 128], BF16)
            nc.vector.tensor_copy(ident_bf[:], ident_f[:])

            # collective bounce buffers
            mx_in = dram.tile([1, C7], F32)
            mx_out = dram.tile([1, C7], F32, addr_space="Shared")
            ms_in = dram.tile([C7, D], F32)
            ms_out = dram.tile([C7, D], F32, addr_space="Shared")
            red_in = dram.tile([C7, 1 + D], F32)
            red_out = dram.tile([C7, 1 + D], F32, addr_space="Shared")
            qt_dram = dram.tile([rt, 128, DK, 128], BF16)

            # ---------------- pass 0: column max of c ----------------
            cmax = cp.tile([128, C7], F32)
            nc.vector.tensor_reduce(out=cmax[:], in_=c_all[:].rearrange("p t j -> p j t"),
                                    axis=mybir.AxisListType.X, op=ALU.max)
            ctp = ps_sm.tile([C7, 128], F32, tag="sm")
            nc.tensor.transpose(ctp[:], cmax[:], ident_f[:])
            mloc = cp.tile([C7, 1], F32)
            nc.vector.tensor_reduce(out=mloc[:], in_=ctp[:], axis=mybir.AxisListType.X,
                                    op=ALU.max)
            nc.sync.dma_start(mx_in[:], mloc[:])
            nc.gpsimd.collective_compute(
                "AllReduce", ALU.max, replica_groups=rg,
                ins=[mx_in[:].opt()], outs=[mx_out[:].opt()])
            mg1 = cp.tile([1, C7], F32)
            nc.sync.dma_start(mg1[:], mx_out[:])
            mg128 = cp.tile([128, C7], F32)
            nc.gpsimd.partition_broadcast(mg128[:], mg1[:])

            # ---------------- helpers ----------------
            def rsqrt_cols(vv, y, t1, t2, p, w):
                """y[:p, :w] = rsqrt(vv[:p, :w]) via bit trick + 2 Newton."""
                nc.vector.tensor_scalar(out=y[:p, :w].bitcast(I32),
                                        in0=vv[:p, :w].bitcast(I32),
                                        scalar1=1, scalar2=-1,
                                        op0=ALU.logical_shift_right,
                                        op1=ALU.bitwise_xor)
                nc.vector.tensor_scalar(out=y[:p, :w].bitcast(I32),
                                        in0=y[:p, :w].bitcast(I32),
                                        scalar1=0x5F3759E0, scalar2=None, op0=ALU.add)
                for _ in range(2):
                    nc.vector.tensor_tensor(out=t1[:p, :w], in0=y[:p, :w],
                                            in1=y[:p, :w], op=ALU.mult)
                    nc.vector.tensor_tensor(out=t2[:p, :w], in0=t1[:p, :w],
                                            in1=vv[:p, :w], op=ALU.mult)
                    nc.vector.tensor_scalar(out=t2[:p, :w], in0=t2[:p, :w], scalar1=-0.5,
                                            scalar2=1.5, op0=ALU.mult, op1=ALU.add)
                    nc.vector.tensor_tensor(out=y[:p, :w], in0=y[:p, :w],
                                            in1=t2[:p, :w], op=ALU.mult)

            def ln_stats(h_ps, mv, p, tagp):
                """mv[:p, 0:2] = (mean, var) of h_ps[:p, :D]."""
                bns = smp.tile([128, 12], F32, tag=f"bns{tagp}")
                nc.vector.bn_stats(bns[:p, 0:6], h_ps[:p, 0:512])
                nc.vector.bn_stats(bns[:p, 6:12], h_ps[:p, 512:1024])
                nc.vector.bn_aggr(mv[:p, :], bns[:p, :])

            def ln_gelu(h_ps, out_ap, p, tagp):
                """out = gelu((h - mean)/sqrt(var+eps)); h_ps [p, D] psum f32."""
                mv = smp.tile([128, 2], F32, tag=f"mv{tagp}")
                vv = smp.tile([128, 1], F32, tag=f"vv{tagp}")
                y = smp.tile([128, 1], F32, tag=f"y{tagp}")
                t1 = smp.tile([128, 1], F32, tag=f"t1{tagp}")
                t2 = smp.tile([128, 1], F32, tag=f"t2{tagp}")
                nb = smp.tile([128, 1], F32, tag=f"nb{tagp}")
                ln_stats(h_ps, mv, p, tagp)
                nc.vector.tensor_scalar(out=vv[:p, :], in0=mv[:p, 1:2], scalar1=EPS,
                                        scalar2=None, op0=ALU.add)
                rsqrt_cols(vv, y, t1, t2, p, 1)
                nc.vector.scalar_tensor_tensor(out=nb[:p, :], in0=mv[:p, 0:1],
                                               scalar=-1.0, in1=y[:p, :],
                                               op0=ALU.mult, op1=ALU.mult)
                nc.scalar.activation(out_ap, h_ps[:p, :], AF.Gelu,
                                     bias=nb[:p, :], scale=y[:p, :])

            def chain_mms(h_ps, lhs_fn, w_sb, b_sb, p):
                """h_ps[:p, :] = lhsT.T @ W + bias (accumulated over DK k-tiles)."""
                for oc in range(2):
                    sl = slice(oc * 512, (oc + 1) * 512)
                    for k in range(DK):
                        nc.tensor.matmul(h_ps[:p, sl], lhs_fn(k), w_sb[k][:, sl],
                                         start=(k == 0), stop=False)
                    nc.tensor.matmul(h_ps[:p, sl],
                                     ones_bf[b_sb.base_partition():b_sb.base_partition() + 1, :p],
                                     b_sb[:1, sl],
                                     start=False, stop=True)

            # shared sequential psum accumulator slot (msel -> hq -> bp)
            msel_ps = ps_acc.tile([C7, D], F32, tag="acc")

            # ---------------- main pass (software-pipelined by 1 tile) ------
            k1_tiles = {}
            k1t_tiles = {}

            def emit_k_chain(t):
                vr = 128  # chains run on full tiles; garbage rows are masked later
                xt_t = iop.tile([128, DK, 128], BF16, tag="xt")
                nc.gpsimd.dma_start(xt_t[:], xt_ext.ap()[t])
                h_ps = ps_h.tile([128, D], F32, tag="h")
                chain_mms(h_ps, lambda k: xt_t[:, k, :], wk_sb, kb_sb, vr)
                k1 = ap_.tile([128, D], BF16, tag="k1")
                ln_gelu(h_ps, k1[:], vr, "m")
                # m_sel accumulation (valid rows only)
                pvr = vr_last if t == rt - 1 else 128
                oh = smp.tile([128, C7], BF16, tag="oh")
                nc.vector.tensor_tensor(out=oh[:], in0=c_all[:, t, :], in1=mg128[:],
                                        op=ALU.is_equal)
                for oc in range(2):
                    sl = slice(oc * 512, (oc + 1) * 512)
                    nc.tensor.matmul(msel_ps[:, sl], oh[:pvr, :], k1[:pvr, sl],
                                     start=(t == 0), stop=(t == rt - 1))
                # K1^T via one batched DMA transpose: k1t[p,k,r] = k1[r, k*128+p]
                k1t = k1tp.tile([128, DK, 128], BF16, tag="k1t")
                nc.sync.dma_start(k1t[:], k1[:], transpose=True)
                k1_tiles[t] = k1
                k1t_tiles[t] = k1t

            def emit_vq_chains(t):
                k1t = k1t_tiles.pop(t)
                k1_tiles.pop(t)
                # V chain
                h_v = ps_h.tile([128, D], F32, tag="h")
                chain_mms(h_v, lambda k: k1t[:, k, :], wv_sb, vb_sb, 128)
                ln_gelu(h_v, v_full[:, t, :], 128, "m")
                # Q chain
                h_q = ps_h.tile([128, D], F32, tag="h")
                chain_mms(h_q, lambda k: k1t[:, k, :], wq_sb, qb_sb, 128)
                q1 = ap_.tile([128, D], BF16, tag="q1")
                ln_gelu(h_q, q1[:], 128, "m")
                qt = ap_.tile([128, DK, 128], BF16, tag="qt")
                nc.sync.dma_start(qt[:], q1[:], transpose=True)
                nc.gpsimd.dma_start(qt_dram[t], qt[:])

            LAG = 1
            for t in range(rt):
                emit_k_chain(t)
                if t >= LAG:
                    emit_vq_chains(t - LAG)
            for t in range(max(0, rt - LAG), rt):
                emit_vq_chains(t)

            # ---------------- m_sel reduce + q_max chain ----------------
            msel_sb = cp.tile([C7, D], F32)
            nc.vector.tensor_copy(msel_sb[:], msel_ps[:])
            nc.sync.dma_start(ms_in[:], msel_sb[:])
            nc.gpsimd.collective_compute(
                "AllReduce", ALU.add, replica_groups=rg,
                ins=[ms_in[:].opt()], outs=[ms_out[:].opt()])
            msel_red = msel_sb
            nc.sync.dma_start(msel_red[:], ms_out[:])

            mselT = cp.tile([128, DK, C7], BF16)
            for k in range(DK):
                tp = ps_sm.tile([128, 128], F32, tag="sm")
                nc.tensor.transpose(tp[:, 0:C7], msel_red[:, k * 128:(k + 1) * 128],
                                    ident_f[0:C7, 0:C7])
                nc.vector.tensor_copy(mselT[:, k, :], tp[:, 0:C7])

            hq_ps = ps_acc.tile([C7, D], F32, tag="acc")
            chain_mms(hq_ps, lambda k: mselT[:, k, :], wq_sb, qb_sb, C7)
            qmax = cp.tile([C7, D], BF16)
            ln_gelu(hq_ps, qmax[:], C7, "q")
            qmaxT = cp.tile([128, DK, C7], BF16)
            for k in range(DK):
                tpb = ps_sm.tile([128, 128], BF16, tag="sm")
                nc.tensor.transpose(tpb[:, 0:C7], qmax[:, k * 128:(k + 1) * 128],
                                    ident_bf[0:C7, 0:C7])
                nc.vector.tensor_copy(qmaxT[:, k, :], tpb[:, 0:C7])

            # ---------------- tail: logits -> exp -> colsum/Bp ----------------
            bp_ps = ps_acc.tile([C7, D], F32, tag="acc")
            cs_acc = cp.tile([C7, 2], F32)
            nc.gpsimd.memset(cs_acc[:], 0.0)
            for t in range(rt):
                pvr = vr_last if t == rt - 1 else 128
                qt_t = iop.tile([128, DK, 128], BF16, tag="qtl")
                nc.gpsimd.dma_start(qt_t[:], qt_dram[t])
                lg = ps_sm.tile([C7, 128], F32, tag="sm")
                for k in range(DK):
                    nc.tensor.matmul(lg[:], qmaxT[:, k, :], qt_t[:, k, :],
                                     start=(k == 0), stop=(k == DK - 1))
                pt_t = smp.tile([C7, 128], BF16, tag="ptt")
                nc.scalar.activation(pt_t[:], lg[:], AF.Exp,
                                     scale=float(1.0 / np.sqrt(D)))
                nc.vector.tensor_reduce(out=cs_acc[:, 1:2], in_=pt_t[:, 0:pvr],
                                        axis=mybir.AxisListType.X, op=ALU.add)
                nc.vector.tensor_tensor(out=cs_acc[:, 0:1], in0=cs_acc[:, 0:1],
                                        in1=cs_acc[:, 1:2], op=ALU.add)
                ptp = ps_sm.tile([128, 128], BF16, tag="sm")
                nc.tensor.transpose(ptp[:, 0:C7], pt_t[:],
                                    ident_bf[0:C7, 0:C7])
                nc.vector.tensor_copy(p_all[:, t, :], ptp[:, 0:C7])
                for oc in range(2):
                    sl = slice(oc * 512, (oc + 1) * 512)
                    nc.tensor.matmul(bp_ps[:, sl], p_all[:pvr, t, :],
                                     v_full[:pvr, t, sl],
                                     start=(t == 0), stop=(t == rt - 1))

            # ---------------- final reduce: [colsum | Bp] ----------------
            red_sb = cp.tile([C7, 1 + D], F32)
            nc.vector.tensor_copy(red_sb[:, 0:1], cs_acc[:, 0:1])
            nc.vector.tensor_copy(red_sb[:, 1:1 + D], bp_ps[:])
            nc.sync.dma_start(red_in[:], red_sb[:])
            nc.gpsimd.collective_compute(
                "AllReduce", ALU.add, replica_groups=rg,
                ins=[red_in[:].opt()], outs=[red_out[:].opt()])
            red2 = red_sb
            nc.sync.dma_start(red2[:], red_out[:])

            # B = Bp / denom (per class)
            rden = cp.tile([C7, 1], F32)
            nc.vector.reciprocal(rden[:], red2[:, 0:1])
            b_sb = cp.tile([C7, D], F32)
            nc.vector.tensor_scalar(out=b_sb[:], in0=red2[:, 1:1 + D], scalar1=rden[:],
                                    scalar2=None, op0=ALU.mult)
            nc.sync.dma_start(b_ext[:, :], b_sb[:])

            # A = P / denom  (broadcast recip over rows)
            den1 = cp.tile([1, C7], F32)
            nc.sync.dma_start(den1[:], red_out[:].rearrange("j o -> o j")[0:1, :])
            rden1 = cp.tile([1, C7], F32)
            nc.vector.reciprocal(rden1[:], den1[:])
            rb128 = cp.tile([128, C7], F32)
            nc.gpsimd.partition_broadcast(rb128[:], rden1[:])
            for t in range(rt):
                pf = smp.tile([128, C7], F32, tag="pf")
                nc.vector.tensor_copy(pf[:], p_all[:, t, :])
                nc.vector.tensor_tensor(out=a_full[:, t, :], in0=pf[:], in1=rb128[:],
                                        op=ALU.mult)
            nc.sync.dma_start(a_ext[:, :, :], a_full[:])

            # C = einsum(B, head_w) + head_b
            btf = cp.tile([128, DK, C7], F32)
            for k in range(DK):
                tp = ps_sm.tile([128, 128], F32, tag="sm")
                nc.tensor.transpose(tp[:, 0:C7], b_sb[:, k * 128:(k + 1) * 128],
                                    ident_f[0:C7, 0:C7])
                nc.vector.tensor_copy(btf[:, k, :], tp[:, 0:C7])
            c_ps = ps_sm.tile([1, C7], F32, tag="sm")
            first = True
            for kb in range(DK):
                for i in range(C7):
                    nc.tensor.matmul(c_ps[:], btf[:, kb, i:i + 1], hwt_sb[:, kb, i, :],
                                     start=first, stop=False)
                    first = False
            nc.tensor.matmul(c_ps[:], one_f[:], hb_sb[:], start=False, stop=True)
            c_sb = cp.tile([1, C7], F32)
            nc.vector.tensor_copy(c_sb[:], c_ps[:])
            nc.sync.dma_start(co_ext[:, :], c_sb[:])

    nc.compile()
    return nc


# --------------------------------------------------------------------------
# host-side prep / unshard
# --------------------------------------------------------------------------

def _prep_core_inputs(shard_x, shard_c, rt):
    """shard_x [rs, 1024] f32, shard_c [rs, 7] f32 -> device layouts."""
    import ml_dtypes
    rs = shard_x.shape[0]
    rpad = rt * 128
    xp = np.zeros((rpad, D), dtype=np.float32)
    xp[:rs] = shard_x
    # [t, p(i_in_tile), k(i_tile), r] ; value = x[t*128+r, k*128+p]
    xt = np.ascontiguousarray(
        xp.reshape(rt, 128, DK, 128).transpose(0, 3, 2, 1)
    ).astype(ml_dtypes.bfloat16)
    cp_ = np.full((rpad, C7), NEG_BIG, dtype=np.float32)
    cp_[:rs] = shard_c
    call = np.ascontiguousarray(cp_.reshape(rt, 128, C7).transpose(1, 0, 2))
    return xt, call


def _prep_shared_inputs(inp):
    import ml_dtypes
    bf = ml_dtypes.bfloat16
    out = {}
    for nm, key in (("wk", "key_w"), ("wv", "value_w"), ("wq", "query_w")):
        w = np.asarray(inp[key], dtype=np.float32)
        out[nm] = np.ascontiguousarray(w.T.reshape(DK, 128, D)).astype(bf)
    for nm, key in (("kb", "key_b"), ("vb", "value_b"), ("qb", "query_b")):
        out[nm] = np.asarray(inp[key], dtype=np.float32).reshape(1, D).astype(bf)
    hw = np.asarray(inp["head_w"], dtype=np.float32)  # [7, 7, 1024]
    out["hwt"] = np.ascontiguousarray(
        hw.transpose(2, 1, 0).reshape(DK, 128, C7, C7).transpose(1, 0, 2, 3)
    )
    out["hb"] = np.asarray(inp["head_b"], dtype=np.float32).reshape(1, C7)
    return out


def kernel(**inputs) -> tuple:
    feats = np.asarray(inputs["features"], dtype=np.float32)
    c = np.asarray(inputs["c"], dtype=np.float32)
    n = feats.shape[0]
    assert n % N_CORES == 0
    rs = n // N_CORES
    rt = (rs + 127) // 128

    key = (rs, N_CORES)
    if key not in _BUILD_CACHE:
        _BUILD_CACHE[key] = build_graph(rs, N_CORES)
    nc = _BUILD_CACHE[key]

    shared = _prep_shared_inputs(inputs)
    in_maps = []
    for s in range(N_CORES):
        xt, call = _prep_core_inputs(feats[s * rs:(s + 1) * rs],
                                     c[s * rs:(s + 1) * rs], rt)
        m = {"xt": xt, "call": call}
        m.update(shared)
        in_maps.append(m)

    res = run_bass_kernel_spmd(nc, in_maps, core_ids=list(range(N_CORES)))

    a_parts = []
    for s in range(N_CORES):
        a_po = res.results[s]["A_out"]                      # [128, rt, 7]
        a_parts.append(a_po.transpose(1, 0, 2).reshape(rt * 128, C7)[:rs])
    A = np.concatenate(a_parts, axis=0).astype(np.float32)
    B = res.results[0]["B_out"].reshape(1, C7, D).astype(np.float32)
    Cc = res.results[0]["C_out"].reshape(1, C7).astype(np.float32)
    return (Cc, A, B)


# revision 19
# speedup vs baseline: 1.3706x; 1.0309x over previous
"""Trainium2 Bass kernel for nn_AttDual: dual-attention MIL pooling head.

Computation (see reference):
  K = chain(features, key_*)    ; chain = Linear -> LayerNorm -> GELU(erf)
  V = chain(K, value_*)
  Q = chain(K, query_*)
  top_idx = argmax(c, axis=0)   ; q_max = chain(K[top_idx], query_*)  (== Q[top_idx])
  A = softmax(Q @ q_max.T / 32, axis=0)   (column softmax over all N)
  B = A.T @ V ;  C = einsum('ik,oik->o', B, head_w) + head_b
Returns (C [1,7], A [N,7], B [1,7,1024]).

Distribution: data-parallel over N across 8 NeuronCores. Three tiny
AllReduces: column max of c (argmax selection), selected K rows, and
[colsum | B] for the softmax normalizer + B.

Matmuls in bf16 with f32 PSUM accumulation. LayerNorm stats via bn_stats;
rsqrt via DVE bit-trick + Newton (keeps ScalarE on the Gelu table the whole
main pass; softmax exp deferred to the tail = 1 table switch total).
"""
import numpy as np

import concourse.bacc as bacc
import concourse.tile as tile
import concourse.mybir as mybir
from concourse.bass_utils import run_bass_kernel_spmd

F32 = mybir.dt.float32
I32 = mybir.dt.int32
BF16 = mybir.dt.bfloat16
AF = mybir.ActivationFunctionType
ALU = mybir.AluOpType

N_CORES = 8
N_TOT = 50000
D = 1024
DK = 8          # d / 128
C7 = 7
EPS = 1e-5
NEG_BIG = -1.0e30

_BUILD_CACHE: dict = {}


# --------------------------------------------------------------------------
# graph builder
# --------------------------------------------------------------------------

def build_graph(rs: int, n_cores: int = N_CORES):
    """rs = rows per core (real). Tiles of 128 rows; last tile partial."""
    rt = (rs + 127) // 128
    vr_last = rs - (rt - 1) * 128

    nc = bacc.Bacc("TRN2", target_bir_lowering=False, debug=False,
                   num_devices=n_cores)

    xt_ext = nc.declare_dram_parameter("xt", [rt, 128, DK, 128], BF16, isOutput=False)
    c_ext = nc.declare_dram_parameter("call", [128, rt, C7], F32, isOutput=False)
    wk_ext = nc.declare_dram_parameter("wk", [DK, 128, D], BF16, isOutput=False)
    wv_ext = nc.declare_dram_parameter("wv", [DK, 128, D], BF16, isOutput=False)
    wq_ext = nc.declare_dram_parameter("wq", [DK, 128, D], BF16, isOutput=False)
    kb_ext = nc.declare_dram_parameter("kb", [1, D], BF16, isOutput=False)
    vb_ext = nc.declare_dram_parameter("vb", [1, D], BF16, isOutput=False)
    qb_ext = nc.declare_dram_parameter("qb", [1, D], BF16, isOutput=False)
    hwt_ext = nc.declare_dram_parameter("hwt", [128, DK, C7, C7], F32, isOutput=False)
    hb_ext = nc.declare_dram_parameter("hb", [1, C7], F32, isOutput=False)

    a_ext = nc.declare_dram_parameter("A_out", [128, rt, C7], F32, isOutput=True)
    b_ext = nc.declare_dram_parameter("B_out", [C7, D], F32, isOutput=True)
    co_ext = nc.declare_dram_parameter("C_out", [1, C7], F32, isOutput=True)

    rg = [list(range(n_cores))]

    with tile.TileContext(nc) as tc:
        with (
            tc.tile_pool(name="const", bufs=1) as cp,
            tc.tile_pool(name="io", bufs=2) as iop,
            tc.tile_pool(name="act", bufs=2) as ap_,
            tc.tile_pool(name="k1tp", bufs=4) as k1tp,
            tc.tile_pool(name="sm", bufs=2) as smp,
            tc.tile_pool(name="ps_h", bufs=2, space="PSUM") as ps_h,
            tc.tile_pool(name="ps_acc", bufs=1, space="PSUM") as ps_acc,
            tc.tile_pool(name="ps_sm", bufs=2, space="PSUM") as ps_sm,
            tc.tile_pool(name="dram", bufs=1, space="DRAM") as dram,
        ):
            # ---------------- constants / pinned ----------------
            c_all = cp.tile([128, rt, C7], F32)
            nc.sync.dma_start(c_all[:], c_ext[:, :, :])
            wk_sb = [cp.tile([128, D], BF16, tag=f"wk{k}", name=f"wk_sb{k}") for k in range(DK)]
            wv_sb = [cp.tile([128, D], BF16, tag=f"wv{k}", name=f"wv_sb{k}") for k in range(DK)]
            wq_sb = [cp.tile([128, D], BF16, tag=f"wq{k}", name=f"wq_sb{k}") for k in range(DK)]
            for k in range(DK):
                nc.scalar.dma_start(wk_sb[k][:], wk_ext.ap()[k])
            for k in range(DK):
                nc.scalar.dma_start(wv_sb[k][:], wv_ext.ap()[k])
            for k in range(DK):
                nc.scalar.dma_start(wq_sb[k][:], wq_ext.ap()[k])
            bias_sb = cp.tile([65, D], BF16)
            nc.scalar.dma_start(bias_sb[0:1, :], kb_ext[:, :])
            nc.scalar.dma_start(bias_sb[32:33, :], vb_ext[:, :])
            nc.scalar.dma_start(bias_sb[64:65, :], qb_ext[:, :])
            kb_sb, vb_sb, qb_sb = bias_sb[0:1, :], bias_sb[32:33, :], bias_sb[64:65, :]
            hwt_sb = cp.tile([128, DK, C7, C7], F32)
            nc.scalar.dma_start(hwt_sb[:], hwt_ext[:, :, :, :])
            hb_sb = cp.tile([1, C7], F32)
            nc.scalar.dma_start(hb_sb[:], hb_ext[:, :])

            v_full = cp.tile([128, rt, D], BF16)
            p_all = cp.tile([128, rt, C7], BF16)
            a_full = cp.tile([128, rt, C7], F32)

            ones_bf = cp.tile([65, 128], BF16)
            nc.gpsimd.memset(ones_bf[:], 1.0)
            ones_col = cp.tile([128, 1], BF16)
            nc.gpsimd.memset(ones_col[:], 1.0)
            one_f = cp.tile([1, 1], F32)
            nc.gpsimd.memset(one_f[:], 1.0)

            # identity matrices (via iota + is_equal)
            iota_p = cp.tile([128, 1], I32)
            nc.gpsimd.iota(iota_p[:], [[0, 1]], channel_multiplier=1)
            iota_pf = cp.tile([128, 1], F32)
            nc.vector.tensor_copy(iota_pf[:], iota_p[:])
            iota_f = cp.tile([128, 128], I32)
            nc.gpsimd.iota(iota_f[:], [[1, 128]], channel_multiplier=0)
            iota_ff = cp.tile([128, 128], F32)
            nc.vector.tensor_copy(iota_ff[:], iota_f[:])
            ident_f = cp.tile([128, 128], F32)
            nc.vector.tensor_scalar(out=ident_f[:], in0=iota_ff[:], scalar1=iota_pf[:],
                                    scalar2=None, op0=ALU.is_equal)
            ident_bf = cp.tile([128, 128], BF16)
            nc.vector.tensor_copy(ident_bf[:], ident_f[:])

            # collective bounce buffers
            mx_in = dram.tile([1, C7], F32)
            mx_out = dram.tile([1, C7], F32, addr_space="Shared")
            ms_in = dram.tile([C7, D], F32)
            ms_out = dram.tile([C7, D], F32, addr_space="Shared")
            red_in = dram.tile([C7, 1 + D], F32)
            red_out = dram.tile([C7, 1 + D], F32, addr_space="Shared")
            qt_dram = dram.tile([rt, 128, DK, 128], BF16)

            # ---------------- pass 0: LOCAL column max of c ----------------
            # one-hot selection uses the core-local max; the global-max
            # AllReduce runs in the background and is only consumed by the
            # tail mask, so it never stalls the PE queue.
            cmax = cp.tile([128, C7], F32)
            nc.vector.tensor_reduce(out=cmax[:], in_=c_all[:].rearrange("p t j -> p j t"),
                                    axis=mybir.AxisListType.X, op=ALU.max)
            ctp = ps_sm.tile([C7, 128], F32, tag="sm")
            nc.tensor.transpose(ctp[:], cmax[:], ident_f[:])
            mloc = cp.tile([C7, 1], F32)
            nc.vector.tensor_reduce(out=mloc[:], in_=ctp[:], axis=mybir.AxisListType.X,
                                    op=ALU.max)
            nc.sync.dma_start(mx_in[:], mloc[:])
            nc.gpsimd.collective_compute(
                "AllReduce", ALU.max, replica_groups=rg,
                ins=[mx_in[:].opt()], outs=[mx_out[:].opt()])
            mltp = ps_sm.tile([C7, 128], F32, tag="sm")
            nc.tensor.transpose(mltp[0:1, 0:C7], mloc[:], ident_f[0:C7, 0:C7])
            ml1 = cp.tile([1, C7], F32)
            nc.vector.tensor_copy(ml1[:], mltp[0:1, 0:C7])
            mg128 = cp.tile([128, C7], F32)
            nc.gpsimd.partition_broadcast(mg128[:], ml1[:])

            # ---------------- helpers ----------------
            def rsqrt_cols(vv, y, t1, t2, p, w):
                """y[:p, :w] = rsqrt(vv[:p, :w]) via bit trick + 2 Newton."""
                nc.vector.tensor_scalar(out=y[:p, :w].bitcast(I32),
                                        in0=vv[:p, :w].bitcast(I32),
                                        scalar1=1, scalar2=-1,
                                        op0=ALU.logical_shift_right,
                                        op1=ALU.bitwise_xor)
                nc.vector.tensor_scalar(out=y[:p, :w].bitcast(I32),
                                        in0=y[:p, :w].bitcast(I32),
                                        scalar1=0x5F3759E0, scalar2=None, op0=ALU.add)
                for _ in range(2):
                    nc.vector.tensor_tensor(out=t1[:p, :w], in0=y[:p, :w],
                                            in1=y[:p, :w], op=ALU.mult)
                    nc.vector.tensor_tensor(out=t2[:p, :w], in0=t1[:p, :w],
                                            in1=vv[:p, :w], op=ALU.mult)
                    nc.vector.tensor_scalar(out=t2[:p, :w], in0=t2[:p, :w], scalar1=-0.5,
                                            scalar2=1.5, op0=ALU.mult, op1=ALU.add)
                    nc.vector.tensor_tensor(out=y[:p, :w], in0=y[:p, :w],
                                            in1=t2[:p, :w], op=ALU.mult)

            def ln_stats(h_ps, mv, p, tagp):
                """mv[:p, 0:2] = (mean, var) of h_ps[:p, :D]."""
                bns = smp.tile([128, 12], F32, tag=f"bns{tagp}")
                nc.vector.bn_stats(bns[:p, 0:6], h_ps[:p, 0:512])
                nc.vector.bn_stats(bns[:p, 6:12], h_ps[:p, 512:1024])
                nc.vector.bn_aggr(mv[:p, :], bns[:p, :])

            def ln_gelu(h_ps, out_ap, p, tagp):
                """out = gelu((h - mean)/sqrt(var+eps)); h_ps [p, D] psum f32."""
                mv = smp.tile([128, 2], F32, tag=f"mv{tagp}")
                vv = smp.tile([128, 1], F32, tag=f"vv{tagp}")
                y = smp.tile([128, 1], F32, tag=f"y{tagp}")
                t1 = smp.tile([128, 1], F32, tag=f"t1{tagp}")
                t2 = smp.tile([128, 1], F32, tag=f"t2{tagp}")
                nb = smp.tile([128, 1], F32, tag=f"nb{tagp}")
                ln_stats(h_ps, mv, p, tagp)
                nc.vector.tensor_scalar(out=vv[:p, :], in0=mv[:p, 1:2], scalar1=EPS,
                                        scalar2=None, op0=ALU.add)
                rsqrt_cols(vv, y, t1, t2, p, 1)
                nc.vector.scalar_tensor_tensor(out=nb[:p, :], in0=mv[:p, 0:1],
                                               scalar=-1.0, in1=y[:p, :],
                                               op0=ALU.mult, op1=ALU.mult)
                nc.scalar.activation(out_ap, h_ps[:p, :], AF.Gelu,
                                     bias=nb[:p, :], scale=y[:p, :])

            def chain_mms(h_ps, lhs_fn, w_sb, b_sb, p):
                """h_ps[:p, :] = lhsT.T @ W + bias (accumulated over DK k-tiles)."""
                for oc in range(2):
                    sl = slice(oc * 512, (oc + 1) * 512)
                    for k in range(DK):
                        nc.tensor.matmul(h_ps[:p, sl], lhs_fn(k), w_sb[k][:, sl],
                                         start=(k == 0), stop=False)
                    nc.tensor.matmul(h_ps[:p, sl],
                                     ones_bf[b_sb.base_partition():b_sb.base_partition() + 1, :p],
                                     b_sb[:1, sl],
                                     start=False, stop=True)

            # shared sequential psum accumulator slot (msel -> hq -> bp)
            msel_ps = ps_acc.tile([C7, D], F32, tag="acc")

            # ---------------- main pass (software-pipelined by 1 tile) ------
            k1_tiles = {}
            k1t_tiles = {}

            def emit_k_chain(t):
                vr = 128  # chains run on full tiles; garbage rows are masked later
                xt_t = iop.tile([128, DK, 128], BF16, tag="xt")
                nc.gpsimd.dma_start(xt_t[:], xt_ext.ap()[t])
                h_ps = ps_h.tile([128, D], F32, tag="h")
                chain_mms(h_ps, lambda k: xt_t[:, k, :], wk_sb, kb_sb, vr)
                k1 = ap_.tile([128, D], BF16, tag="k1")
                ln_gelu(h_ps, k1[:], vr, "m")
                # m_sel accumulation (valid rows only)
                pvr = vr_last if t == rt - 1 else 128
                oh = smp.tile([128, C7], BF16, tag="oh")
                nc.vector.tensor_tensor(out=oh[:], in0=c_all[:, t, :], in1=mg128[:],
                                        op=ALU.is_equal)
                for oc in range(2):
                    sl = slice(oc * 512, (oc + 1) * 512)
                    nc.tensor.matmul(msel_ps[:, sl], oh[:pvr, :], k1[:pvr, sl],
                                     start=(t == 0), stop=(t == rt - 1))
                # K1^T via one batched DMA transpose: k1t[p,k,r] = k1[r, k*128+p]
                k1t = k1tp.tile([128, DK, 128], BF16, tag="k1t")
                nc.sync.dma_start(k1t[:], k1[:], transpose=True)
                k1_tiles[t] = k1
                k1t_tiles[t] = k1t

            def emit_vq_chains(t):
                k1t = k1t_tiles.pop(t)
                k1_tiles.pop(t)
                # V chain
                h_v = ps_h.tile([128, D], F32, tag="h")
                chain_mms(h_v, lambda k: k1t[:, k, :], wv_sb, vb_sb, 128)
                ln_gelu(h_v, v_full[:, t, :], 128, "m")
                # Q chain
                h_q = ps_h.tile([128, D], F32, tag="h")
                chain_mms(h_q, lambda k: k1t[:, k, :], wq_sb, qb_sb, 128)
                q1 = ap_.tile([128, D], BF16, tag="q1")
                ln_gelu(h_q, q1[:], 128, "m")
                qt = ap_.tile([128, DK, 128], BF16, tag="qt")
                nc.sync.dma_start(qt[:], q1[:], transpose=True)
                nc.gpsimd.dma_start(qt_dram[t], qt[:])

            LAG = 2
            for t in range(rt):
                emit_k_chain(t)
                if t >= LAG:
                    emit_vq_chains(t - LAG)
            for t in range(max(0, rt - LAG), rt):
                emit_vq_chains(t)

            # ---------------- m_sel reduce + q_max chain ----------------
            msel_sb = cp.tile([C7, D], F32)
            mg7 = cp.tile([C7, 1], F32)
            nc.sync.dma_start(mg7[:], mx_out[:].rearrange("o j -> j o"))
            mask7 = cp.tile([C7, 1], F32)
            nc.vector.tensor_tensor(out=mask7[:], in0=mloc[:], in1=mg7[:],
                                    op=ALU.is_equal)
            nc.vector.tensor_scalar(out=msel_sb[:], in0=msel_ps[:], scalar1=mask7[:],
                                    scalar2=None, op0=ALU.mult)
            nc.sync.dma_start(ms_in[:], msel_sb[:])
            nc.gpsimd.collective_compute(
                "AllReduce", ALU.add, replica_groups=rg,
                ins=[ms_in[:].opt()], outs=[ms_out[:].opt()])
            msel_red = msel_sb
            nc.sync.dma_start(msel_red[:], ms_out[:])

            mselT = cp.tile([128, DK, C7], BF16)
            for k in range(DK):
                tp = ps_sm.tile([128, 128], F32, tag="sm")
                nc.tensor.transpose(tp[:, 0:C7], msel_red[:, k * 128:(k + 1) * 128],
                                    ident_f[0:C7, 0:C7])
                nc.vector.tensor_copy(mselT[:, k, :], tp[:, 0:C7])

            hq_ps = ps_acc.tile([C7, D], F32, tag="acc")
            chain_mms(hq_ps, lambda k: mselT[:, k, :], wq_sb, qb_sb, C7)
            qmax = cp.tile([C7, D], BF16)
            ln_gelu(hq_ps, qmax[:], C7, "q")
            qmaxT = cp.tile([128, DK, C7], BF16)
            for k in range(DK):
                tpb = ps_sm.tile([128, 128], BF16, tag="sm")
                nc.tensor.transpose(tpb[:, 0:C7], qmax[:, k * 128:(k + 1) * 128],
                                    ident_bf[0:C7, 0:C7])
                nc.vector.tensor_copy(qmaxT[:, k, :], tpb[:, 0:C7])

            # ---------------- tail: logits -> exp -> colsum/Bp ----------------
            bp_ps = ps_acc.tile([C7, D], F32, tag="acc")
            cs_acc = cp.tile([C7, 2], F32)
            nc.gpsimd.memset(cs_acc[:], 0.0)
            for t in range(rt):
                pvr = vr_last if t == rt - 1 else 128
                qt_t = iop.tile([128, DK, 128], BF16, tag="qtl")
                nc.gpsimd.dma_start(qt_t[:], qt_dram[t])
                lg = ps_sm.tile([C7, 128], F32, tag="sm")
                for k in range(DK):
                    nc.tensor.matmul(lg[:], qmaxT[:, k, :], qt_t[:, k, :],
                                     start=(k == 0), stop=(k == DK - 1))
                pt_t = smp.tile([C7, 128], BF16, tag="ptt")
                nc.scalar.activation(pt_t[:], lg[:], AF.Exp,
                                     scale=float(1.0 / np.sqrt(D)))
                nc.vector.tensor_reduce(out=cs_acc[:, 1:2], in_=pt_t[:, 0:pvr],
                                        axis=mybir.AxisListType.X, op=ALU.add)
                nc.vector.tensor_tensor(out=cs_acc[:, 0:1], in0=cs_acc[:, 0:1],
                                        in1=cs_acc[:, 1:2], op=ALU.add)
                ptp = ps_sm.tile([128, 128], BF16, tag="sm")
                nc.tensor.transpose(ptp[:, 0:C7], pt_t[:],
                                    ident_bf[0:C7, 0:C7])
                nc.vector.tensor_copy(p_all[:, t, :], ptp[:, 0:C7])
                for oc in range(2):
                    sl = slice(oc * 512, (oc + 1) * 512)
                    nc.tensor.matmul(bp_ps[:, sl], p_all[:pvr, t, :],
                                     v_full[:pvr, t, sl],
                                     start=(t == 0), stop=(t == rt - 1))

            # ---------------- final reduce: [colsum | Bp] ----------------
            red_sb = cp.tile([C7, 1 + D], F32)
            nc.vector.tensor_copy(red_sb[:, 0:1], cs_acc[:, 0:1])
            nc.vector.tensor_copy(red_sb[:, 1:1 + D], bp_ps[:])
            nc.sync.dma_start(red_in[:], red_sb[:])
            nc.gpsimd.collective_compute(
                "AllReduce", ALU.add, replica_groups=rg,
                ins=[red_in[:].opt()], outs=[red_out[:].opt()])
            red2 = red_sb
            nc.sync.dma_start(red2[:], red_out[:])

            # B = Bp / denom (per class)
            rden = cp.tile([C7, 1], F32)
            nc.vector.reciprocal(rden[:], red2[:, 0:1])
            b_sb = cp.tile([C7, D], F32)
            nc.vector.tensor_scalar(out=b_sb[:], in0=red2[:, 1:1 + D], scalar1=rden[:],
                                    scalar2=None, op0=ALU.mult)
            nc.sync.dma_start(b_ext[:, :], b_sb[:])

            # A = P / denom  (broadcast recip over rows)
            den1 = cp.tile([1, C7], F32)
            nc.sync.dma_start(den1[:], red_out[:].rearrange("j o -> o j")[0:1, :])
            rden1 = cp.tile([1, C7], F32)
            nc.vector.reciprocal(rden1[:], den1[:])
            rb128 = cp.tile([128, C7], F32)
            nc.gpsimd.partition_broadcast(rb128[:], rden1[:])
            for t in range(rt):
                pf = smp.tile([128, C7], F32, tag="pf")
                nc.vector.tensor_copy(pf[:], p_all[:, t, :])
                nc.vector.tensor_tensor(out=a_full[:, t, :], in0=pf[:], in1=rb128[:],
                                        op=ALU.mult)
            nc.sync.dma_start(a_ext[:, :, :], a_full[:])

            # C = einsum(B, head_w) + head_b
            btf = cp.tile([128, DK, C7], F32)
            for k in range(DK):
                tp = ps_sm.tile([128, 128], F32, tag="sm")
                nc.tensor.transpose(tp[:, 0:C7], b_sb[:, k * 128:(k + 1) * 128],
                                    ident_f[0:C7, 0:C7])
                nc.vector.tensor_copy(btf[:, k, :], tp[:, 0:C7])
            c_ps = ps_sm.tile([1, C7], F32, tag="sm")
            first = True
            for kb in range(DK):
                for i in range(C7):
                    nc.tensor.matmul(c_ps[:], btf[:, kb, i:i + 1], hwt_sb[:, kb, i, :],
                                     start=first, stop=False)
                    first = False
            nc.tensor.matmul(c_ps[:], one_f[:], hb_sb[:], start=False, stop=True)
            c_sb = cp.tile([1, C7], F32)
            nc.vector.tensor_copy(c_sb[:], c_ps[:])
            nc.sync.dma_start(co_ext[:, :], c_sb[:])

    nc.compile()
    return nc


# --------------------------------------------------------------------------
# host-side prep / unshard
# --------------------------------------------------------------------------

def _prep_core_inputs(shard_x, shard_c, rt):
    """shard_x [rs, 1024] f32, shard_c [rs, 7] f32 -> device layouts."""
    import ml_dtypes
    rs = shard_x.shape[0]
    rpad = rt * 128
    xp = np.zeros((rpad, D), dtype=np.float32)
    xp[:rs] = shard_x
    # [t, p(i_in_tile), k(i_tile), r] ; value = x[t*128+r, k*128+p]
    xt = np.ascontiguousarray(
        xp.reshape(rt, 128, DK, 128).transpose(0, 3, 2, 1)
    ).astype(ml_dtypes.bfloat16)
    cp_ = np.full((rpad, C7), NEG_BIG, dtype=np.float32)
    cp_[:rs] = shard_c
    call = np.ascontiguousarray(cp_.reshape(rt, 128, C7).transpose(1, 0, 2))
    return xt, call


def _prep_shared_inputs(inp):
    import ml_dtypes
    bf = ml_dtypes.bfloat16
    out = {}
    for nm, key in (("wk", "key_w"), ("wv", "value_w"), ("wq", "query_w")):
        w = np.asarray(inp[key], dtype=np.float32)
        out[nm] = np.ascontiguousarray(w.T.reshape(DK, 128, D)).astype(bf)
    for nm, key in (("kb", "key_b"), ("vb", "value_b"), ("qb", "query_b")):
        out[nm] = np.asarray(inp[key], dtype=np.float32).reshape(1, D).astype(bf)
    hw = np.asarray(inp["head_w"], dtype=np.float32)  # [7, 7, 1024]
    out["hwt"] = np.ascontiguousarray(
        hw.transpose(2, 1, 0).reshape(DK, 128, C7, C7).transpose(1, 0, 2, 3)
    )
    out["hb"] = np.asarray(inp["head_b"], dtype=np.float32).reshape(1, C7)
    return out


def kernel(**inputs) -> tuple:
    feats = np.asarray(inputs["features"], dtype=np.float32)
    c = np.asarray(inputs["c"], dtype=np.float32)
    n = feats.shape[0]
    assert n % N_CORES == 0
    rs = n // N_CORES
    rt = (rs + 127) // 128

    key = (rs, N_CORES)
    if key not in _BUILD_CACHE:
        _BUILD_CACHE[key] = build_graph(rs, N_CORES)
    nc = _BUILD_CACHE[key]

    shared = _prep_shared_inputs(inputs)
    in_maps = []
    for s in range(N_CORES):
        xt, call = _prep_core_inputs(feats[s * rs:(s + 1) * rs],
                                     c[s * rs:(s + 1) * rs], rt)
        m = {"xt": xt, "call": call}
        m.update(shared)
        in_maps.append(m)

    res = run_bass_kernel_spmd(nc, in_maps, core_ids=list(range(N_CORES)))

    a_parts = []
    for s in range(N_CORES):
        a_po = res.results[s]["A_out"]                      # [128, rt, 7]
        a_parts.append(a_po.transpose(1, 0, 2).reshape(rt * 128, C7)[:rs])
    A = np.concatenate(a_parts, axis=0).astype(np.float32)
    B = res.results[0]["B_out"].reshape(1, C7, D).astype(np.float32)
    Cc = res.results[0]["C_out"].reshape(1, C7).astype(np.float32)
    return (Cc, A, B)
